# revision 12
# baseline (speedup 1.0000x reference)
# Trainium2 Bass kernel for nn_DLSMNLayer (DLSMN layer: cache cross-attention,
# gated fusion, transformer block, pattern pooling, selection head).
#
# Strategy: data-parallel over batch B=8 across the 8 NeuronCores (one batch
# element per core). Inside each core the whole layer runs fused in SBUF:
#   - activations are kept feature-major ("transposed", [feature, token]) so
#     every linear layer is a plain PE matmul with host-pretransposed weights
#   - matmuls run in bf16 with fp32 PSUM accumulation; the residual stream
#     (xf, preln1, xf1, preln2) and both layernorm applications stay fp32
#   - attention is computed transpose-free: scores are produced as S^T
#     [keys, queries], softmax denominators come from an extra ones-column in
#     the value matrix (one PE accumulation), normalization folds into the
#     PSUM eviction
#   - per-token stats (layernorm mean/var, softmax sums) that live across the
#     partition axis are computed with ones-vector PE matmuls and broadcast
#     back across partitions with k=1 PE matmuls / gpsimd partition_broadcast
#   - SBUF tiles pad to 4KB/partition, so small constants are packed into
#     shared "bank" tiles and pools are scoped tightly to stages
#
# kernel(**inputs) takes the full unsharded inputs and returns the same
# 6-tuple as the reference: (y, patterns, scores, slot_probs, soft_probs, alpha)

import contextlib

import numpy as np
import ml_dtypes

import concourse.bacc as bacc
import concourse.bass as bass
import concourse.tile as tile
import concourse.mybir as mybir
from concourse import bass_utils
from concourse.masks import make_identity

F32 = mybir.dt.float32
BF16 = mybir.dt.bfloat16
AF = mybir.ActivationFunctionType
OP = mybir.AluOpType
BFNP = ml_dtypes.bfloat16

D = 1024      # d_model
DC = 256      # d_cache
DL = 64       # d_layer
KSL = 64      # num_slots
NCA = 512     # cache slots
PP = 16       # num_patterns
H = 16        # num_heads
DH = 64       # head dim
B, S = 8, 1024
FF = 4096
NCORES = 8
HALF = 512
HALVES = (0, HALF)

_CACHE = {}

# bias bank column map (f32 [128, 280])
BB = dict(cqb=0, ckb=2, decb=4, fgb=12, bqk=20, wob=36, n1g=44, n1b=52,
          n2g=60, n2b=68, b1=76, b2=108, pqb=116, pkb=124, pwob=132,
          sqb=140, skb=142, selb=144, anb=145, gateb=146, eps=147, onesr=152)
BBW = 280
# bf16 row bank column map ([1, 2304])
RB = dict(cvb=0, bv=256, pvb=1280)


# ----------------------------------------------------------------------------
# host-side weight layout helpers
# ----------------------------------------------------------------------------

def _pcn(wT, dtype=BFNP):
    """[K, M] -> [128, K//128, M]: partition p, chunk c <-> row c*128+p."""
    K, M = wT.shape
    assert K % 128 == 0
    return np.ascontiguousarray(
        np.asarray(wT).reshape(K // 128, 128, M).transpose(1, 0, 2).astype(dtype))


def _blk(wT, dtype=BFNP):
    """[K, M] -> [M//128, 128, K//128, 128] contiguous m-blocks in pcn layout."""
    K, M = wT.shape
    assert K % 128 == 0 and M % 128 == 0
    return np.ascontiguousarray(
        np.asarray(wT).reshape(K // 128, 128, M // 128, 128)
        .transpose(2, 1, 0, 3).astype(dtype))


def _bias_t(b):
    """[L] -> [128, L//128]: column m holds bias for features m*128..m*128+127."""
    L = b.shape[0]
    return np.ascontiguousarray(np.asarray(b, np.float32).reshape(L // 128, 128).T)


def _pad_rows(a, K):
    out = np.zeros((K, a.shape[1]), a.dtype)
    out[: a.shape[0]] = a
    return out


# ----------------------------------------------------------------------------
# device kernel
# ----------------------------------------------------------------------------

def _declare_tensors(nc):
    T = {}

    def di(name, shape, dt):
        T[name] = nc.dram_tensor(name, list(shape), dt, kind="ExternalInput").ap()

    def do(name, shape, dt):
        T[name] = nc.dram_tensor(name, list(shape), dt, kind="ExternalOutput").ap()

    di("xT", (D, S), F32)
    di("cid_pcn", (128, 3, NCA), BF16)
    di("gT", (KSL, PP), F32)

    di("biasbank", (128, BBW), F32)
    di("rowbank", (1, 2304), BF16)

    di("cqw_blk", (2, 128, 8, 128), BF16)
    di("ckw_pcn", (128, 3, DC), BF16)
    di("cvw_pcn", (128, 3, DC), BF16)
    di("decw_blk", (8, 128, 2, 128), BF16)
    di("fgw_blk", (8, 128, 16, 128), BF16)
    di("wqk_blk", (16, 128, 8, 128), BF16)
    di("wv_pcn", (128, 8, D), BF16)
    di("wo_blk", (8, 128, 8, 128), BF16)
    di("w1_blk", (32, 128, 8, 128), BF16)
    di("w2_blk", (8, 128, 32, 128), BF16)
    di("pq_blk", (8, 128, 8, 128), BF16)
    di("pk_blk", (8, 128, 8, 128), BF16)
    di("pv_pcn", (128, 8, D), BF16)
    di("pwo_blk", (8, 128, 8, 128), BF16)
    di("queries_pcn", (128, 8, PP), BF16)
    di("selw_pcn", (128, 8, KSL), BF16)
    di("sqw_pcn", (128, 8, DC), BF16)
    di("skw_pcn", (128, 2, DC), BF16)
    di("slotemb_pcn", (128, 2, KSL), BF16)
    di("anw_pcn", (128, 8, 1), BF16)
    di("gatew_pcn", (128, 8, 1), BF16)

    do("yT_out", (D, S), F32)
    do("patsT_out", (8, 128, PP), F32)
    do("scores_out", (1, PP), F32)
    do("alpha_out", (1, PP), F32)
    do("slotp_out", (PP, KSL), F32)
    do("softp_out", (PP, KSL), F32)
    return T


def _emit(nc, tc, T):
    with contextlib.ExitStack() as ctx:
        # ---------------- constants (packed banks) ----------------
        const = ctx.enter_context(tc.tile_pool(name="const", bufs=1))
        onesall = const.tile([128, 128], BF16, tag="onesall")
        nc.vector.memset(onesall, 1.0)
        ones1 = onesall[0:1, :]          # [1,128] row of ones (bias-row matmuls)
        onesd = onesall[:, 0:1]          # [128,1] column of ones (LN sums)
        bank = const.tile([128, BBW], F32, tag="bank")
        nc.sync.dma_start(out=bank, in_=T["biasbank"])
        rowb = const.tile([1, 2304], BF16, tag="rowb")
        nc.sync.dma_start(out=rowb, in_=T["rowbank"])

        def bb(key, m):
            return bank[:, BB[key] + m:BB[key] + m + 1]

        onesr = bank[0:1, BB["onesr"]:BB["onesr"] + 128]   # [1,128] f32 ones
        eps_ap = bank[0:1, BB["eps"]:BB["eps"] + 1]

        # ---------------- long-lived pools ----------------
        actp = ctx.enter_context(tc.tile_pool(name="actp", bufs=4))    # 16KB tiles
        big4 = tc.alloc_tile_pool(name="big4", bufs=2)                 # 32KB f32 tiles

        # generic blocked projection
        def proj(w_blk_ap, ms, kc, rhs_sel, evict, wpool, wtag, pspool, pstag,
                 nfree=S, nstep=HALF, extra_mm=None):
            for mi, m in enumerate(ms):
                wt = wpool.tile([128, kc, 128], BF16, tag=wtag, name=f"w{wtag}{m}")
                nc.sync.dma_start(out=wt, in_=w_blk_ap[m])
                for h0 in range(0, nfree, nstep):
                    ps = pspool.tile([128, nstep], F32, tag=pstag, name=f"ps{wtag}")
                    for c in range(kc):
                        nc.tensor.matmul(ps, lhsT=wt[:, c, :],
                                         rhs=rhs_sel(c, h0, nstep),
                                         start=(c == 0),
                                         stop=(extra_mm is None and c == kc - 1))
                    if extra_mm is not None:
                        extra_mm(ps, h0, nstep)
                    evict(mi, m, h0, ps)

        xf_f32 = big4.tile([128, 8, S], F32, tag="a4", name="xf_f32")
        xf_bf = actp.tile([128, 8, S], BF16, tag="a2", name="xf_bf")

        # ==================================================================
        # Stage A: read_cache cross-attention + decode + gated fusion
        # ==================================================================
        with tc.tile_pool(name="pa", bufs=1) as pa, \
             tc.tile_pool(name="pa2", bufs=2) as pa2, \
             tc.tile_pool(name="psAmm", bufs=2, space="PSUM") as psmm:

            xT_bf = pa.tile([128, 8, S], BF16, tag="xT_bf")
            for c in range(8):
                xs = pa2.tile([128, S], F32, tag="xld", name=f"xld{c}")
                nc.sync.dma_start(out=xs, in_=T["xT"][c * 128:(c + 1) * 128, :])
                nc.scalar.copy(out=xT_bf[:, c, :], in_=xs)
            ctxT = pa.tile([128, 2, S], BF16, tag="ctxT")

            ca = tc.alloc_tile_pool(name="ca", bufs=1)
            psvv = tc.alloc_tile_pool(name="psAvv", bufs=2, space="PSUM")
            pssum = tc.alloc_tile_pool(name="psAsum", bufs=2, space="PSUM")
            psctx = tc.alloc_tile_pool(name="psActx", bufs=2, space="PSUM")

            cid = ca.tile([128, 3, NCA], BF16, tag="cid")
            nc.sync.dma_start(out=cid, in_=T["cid_pcn"])
            ckcv = ca.tile([128, 3, 2 * DC], BF16, tag="ckcv")
            nc.sync.dma_start(out=ckcv[:, :, 0:DC], in_=T["ckw_pcn"])
            nc.sync.dma_start(out=ckcv[:, :, DC:2 * DC], in_=T["cvw_pcn"])

            # q_ca = (x @ cq_w.T + cq_b)/16
            qca = ca.tile([128, 2, S], BF16, tag="qca")
            def ev_qca(mi, m, h0, ps):
                nc.vector.tensor_scalar(out=qca[:, m, h0:h0 + HALF], in0=ps,
                                        scalar1=bb("cqb", m), scalar2=1.0 / 16.0,
                                        op0=OP.add, op1=OP.mult)
            proj(T["cqw_blk"], range(2), 8, lambda c, h0, n: xT_bf[:, c, h0:h0 + n],
                 ev_qca, pa2, "wA", psmm, "mm")

            # kk^T [dc, n]
            kkT = ca.tile([128, 2, NCA], BF16, tag="kkT")
            for m in range(2):
                ps = psmm.tile([128, NCA], F32, tag="mm", name=f"pskk{m}")
                for c in range(3):
                    nc.tensor.matmul(ps, lhsT=ckcv[:, c, m * 128:(m + 1) * 128],
                                     rhs=cid[:, c, :], start=(c == 0), stop=(c == 2))
                nc.vector.tensor_scalar(out=kkT[:, m, :], in0=ps,
                                        scalar1=bb("ckb", m), scalar2=None, op0=OP.add)

            # vv natural [n, dc] + ones column
            vvaug = ca.tile([128, 4, DC + 1], BF16, tag="vvaug")
            for nt in range(4):
                ps = psvv.tile([128, DC], F32, tag="vv", name=f"psvv{nt}")
                for c in range(3):
                    nc.tensor.matmul(ps, lhsT=cid[:, c, nt * 128:(nt + 1) * 128],
                                     rhs=ckcv[:, c, DC:2 * DC], start=(c == 0), stop=False)
                nc.tensor.matmul(ps, lhsT=ones1, rhs=rowb[:, RB["cvb"]:RB["cvb"] + DC],
                                 start=False, stop=True)
                nc.vector.tensor_copy(out=vvaug[:, nt, 0:DC], in_=ps)
                nc.vector.memset(vvaug[:, nt, DC:DC + 1], 1.0)

            # probs^T = exp(kk q / 16)
            pca = ca.tile([128, 4, S], BF16, tag="pca")
            for nt in range(4):
                for h0 in HALVES:
                    ps = psmm.tile([128, HALF], F32, tag="mm", name=f"pssc{nt}")
                    for c in range(2):
                        nc.tensor.matmul(ps, lhsT=kkT[:, c, nt * 128:(nt + 1) * 128],
                                         rhs=qca[:, c, h0:h0 + HALF],
                                         start=(c == 0), stop=(c == 1))
                    nc.scalar.activation(out=pca[:, nt, h0:h0 + HALF], in_=ps, func=AF.Exp)

            # ctx^T [dc, s] with ones-column sums
            recip_ca = ca.tile([1, S], F32, tag="recip_ca")
            for h0 in HALVES:
                pss = pssum.tile([1, HALF], F32, tag="casum", name="pscas")
                for nt in range(4):
                    nc.tensor.matmul(pss, lhsT=vvaug[:, nt, DC:DC + 1],
                                     rhs=pca[:, nt, h0:h0 + HALF],
                                     start=(nt == 0), stop=(nt == 3))
                nc.vector.reciprocal(out=recip_ca[:, h0:h0 + HALF], in_=pss)
            recb_ca = ca.tile([128, S], F32, tag="recb_ca")
            nc.gpsimd.partition_broadcast(recb_ca, recip_ca, channels=128)

            for m in range(2):
                for h0 in HALVES:
                    ps = psctx.tile([128, HALF], F32, tag="ctx", name="psctx")
                    for nt in range(4):
                        nc.tensor.matmul(ps, lhsT=vvaug[:, nt, m * 128:(m + 1) * 128],
                                         rhs=pca[:, nt, h0:h0 + HALF],
                                         start=(nt == 0), stop=(nt == 3))
                    nc.vector.tensor_tensor(out=ctxT[:, m, h0:h0 + HALF], in0=ps,
                                            in1=recb_ca[:, h0:h0 + HALF], op=OP.mult)
            ca.release()
            psctx.release()
            pssum.release()
            psvv.release()

            # context^T = dec_w[:, :256] @ ctx^T + decb_eff (layer_embed folded)
            context_bf = actp.tile([128, 8, S], BF16, tag="a2", name="context_bf")
            def ev_dec(mi, m, h0, ps):
                nc.vector.tensor_scalar(out=context_bf[:, m, h0:h0 + HALF], in0=ps,
                                        scalar1=bb("decb", m), scalar2=None, op0=OP.add)
            proj(T["decw_blk"], range(8), 2, lambda c, h0, n: ctxT[:, c, h0:h0 + n],
                 ev_dec, pa2, "wA", psmm, "mm")

            # gated fusion -> xf (fp32) + xf_bf
            pg = tc.alloc_tile_pool(name="pg", bufs=2)
            def fg_rhs(c, h0, n):
                return (xT_bf if c < 8 else context_bf)[:, c % 8, h0:h0 + n]
            def ev_fg(mi, m, h0, ps):
                g = pg.tile([128, HALF], F32, tag="gsig", name="gsig")
                nc.scalar.activation(out=g, in_=ps, func=AF.Sigmoid, bias=bb("fgb", m))
                dd = pg.tile([128, HALF], F32, tag="gd", name="gd")
                nc.vector.tensor_tensor(out=dd, in0=xT_bf[:, m, h0:h0 + HALF],
                                        in1=context_bf[:, m, h0:h0 + HALF], op=OP.subtract)
                nc.vector.tensor_tensor(out=dd, in0=dd, in1=g, op=OP.mult)
                nc.vector.tensor_tensor(out=xf_f32[:, m, h0:h0 + HALF], in0=dd,
                                        in1=context_bf[:, m, h0:h0 + HALF], op=OP.add)
                nc.scalar.copy(out=xf_bf[:, m, h0:h0 + HALF],
                               in_=xf_f32[:, m, h0:h0 + HALF])
            proj(T["fgw_blk"], range(8), 16, fg_rhs, ev_fg, pa2, "wA", psmm, "mm")
            pg.release()

        # ==================================================================
        # Stage B: self-attention block
        # ==================================================================
        preln1 = big4.tile([128, 8, S], F32, tag="a4", name="preln1")
        o_bf = actp.tile([128, 8, S], BF16, tag="a2", name="o_bf")
        with tc.tile_pool(name="pb", bufs=1) as pb, \
             tc.tile_pool(name="pb2", bufs=2) as pb2, \
             tc.tile_pool(name="probsp", bufs=2) as probsp, \
             tc.tile_pool(name="psBmm", bufs=2, space="PSUM") as psmm, \
             tc.tile_pool(name="psBsc", bufs=3, space="PSUM") as pssc, \
             tc.tile_pool(name="psBo", bufs=2, space="PSUM") as pso_p:

            # v natural with per-head ones columns (half-major so wv streams once)
            vaug = pb.tile([128, 8, H * (DH + 1)], BF16, tag="vaug")
            for hi, h0 in enumerate(HALVES):
                wvh = pb2.tile([128, 8, HALF], BF16, tag="wvh", name=f"wvh{hi}", bufs=1)
                nc.sync.dma_start(out=wvh, in_=T["wv_pcn"][:, :, h0:h0 + HALF])
                for st in range(8):
                    ps = psmm.tile([128, HALF], F32, tag="mm", name="psv")
                    for c in range(8):
                        nc.tensor.matmul(ps, lhsT=xf_bf[:, c, st * 128:(st + 1) * 128],
                                         rhs=wvh[:, c, :], start=(c == 0), stop=False)
                    nc.tensor.matmul(ps, lhsT=ones1,
                                     rhs=rowb[:, RB["bv"] + h0:RB["bv"] + h0 + HALF],
                                     start=False, stop=True)
                    nc.vector.tensor_copy(
                        out=vaug[:, st, :].rearrange("p (h e) -> p h e", e=DH + 1)
                        [:, hi * 8:(hi + 1) * 8, 0:DH],
                        in_=ps.rearrange("p (h e) -> p h e", e=DH))
            for st in range(8):
                nc.vector.memset(
                    vaug[:, st, :].rearrange("p (h e) -> p h e", e=DH + 1)[:, :, DH:DH + 1],
                    1.0)

            # q/k head groups of 4 heads (2 feature chunks each)
            for grp in range(4):
                qg = pb2.tile([128, 2, S], BF16, tag="qg", name=f"qg{grp}")
                kg = pb2.tile([128, 2, S], BF16, tag="kg", name=f"kg{grp}")
                def ev_qk(mi, m, h0, ps, qg=qg, kg=kg):
                    if m < 8:
                        nc.vector.tensor_scalar(out=qg[:, mi, h0:h0 + HALF], in0=ps,
                                                scalar1=bb("bqk", m), scalar2=1.0 / 8.0,
                                                op0=OP.add, op1=OP.mult)
                    else:
                        nc.vector.tensor_scalar(out=kg[:, mi - 2, h0:h0 + HALF], in0=ps,
                                                scalar1=bb("bqk", m), scalar2=None,
                                                op0=OP.add)
                proj(T["wqk_blk"], [2 * grp, 2 * grp + 1, 8 + 2 * grp, 9 + 2 * grp], 8,
                     lambda c, h0, n: xf_bf[:, c, h0:h0 + n],
                     ev_qk, pb2, "wB", psmm, "mm")

                for hh in range(4):
                    h = grp * 4 + hh
                    hp = (hh % 2) * 64
                    hc = hh // 2
                    for h0 in HALVES:
                        probs = probsp.tile([128, 8, HALF], BF16, tag="probs",
                                            name=f"probs{h}")
                        for kt in range(8):
                            ps = pssc.tile([128, HALF], F32, tag="sc", name="pssc")
                            nc.tensor.matmul(
                                ps, lhsT=kg[hp:hp + 64, hc, kt * 128:(kt + 1) * 128],
                                rhs=qg[hp:hp + 64, hc, h0:h0 + HALF],
                                start=True, stop=True)
                            nc.scalar.activation(out=probs[:, kt, :], in_=ps, func=AF.Exp)
                        pso = pso_p.tile([DH + 1, HALF], F32, tag="o", name="pso")
                        for kt in range(8):
                            nc.tensor.matmul(pso,
                                             lhsT=vaug[:, kt, h * (DH + 1):(h + 1) * (DH + 1)],
                                             rhs=probs[:, kt, :],
                                             start=(kt == 0), stop=(kt == 7))
                        rec = pb2.tile([1, HALF], F32, tag="rec", name="rec")
                        nc.vector.reciprocal(out=rec, in_=pso[DH:DH + 1, :])
                        recb = pb2.tile([64, HALF], F32, tag="recb", name="recb")
                        nc.gpsimd.partition_broadcast(recb, rec, channels=64)
                        nc.vector.tensor_tensor(
                            out=o_bf[(h % 2) * 64:(h % 2) * 64 + 64, h // 2, h0:h0 + HALF],
                            in0=pso[0:DH, :], in1=recb, op=OP.mult)

            # out proj + residual -> preln1 (fp32)
            def ev_wo(mi, m, h0, ps):
                nc.vector.scalar_tensor_tensor(out=preln1[:, m, h0:h0 + HALF], in0=ps,
                                               scalar=bb("wob", m),
                                               in1=xf_f32[:, m, h0:h0 + HALF],
                                               op0=OP.add, op1=OP.add)
            proj(T["wo_blk"], range(8), 8, lambda c, h0, n: o_bf[:, c, h0:h0 + n],
                 ev_wo, pb2, "wB", psmm, "mm")

        # ==================================================================
        # layernorm (feature axis = partitions, PE-based stats)
        # ==================================================================
        def emit_ln(pre_f32, gkey, bkey, write_out, lnname):
            with tc.tile_pool(name=f"{lnname}s", bufs=2) as lps, \
                 tc.tile_pool(name=f"{lnname}st", bufs=1) as lst, \
                 tc.tile_pool(name=f"{lnname}ps", bufs=1, space="PSUM") as psln:
                ps_mean = psln.tile([1, S], F32, tag="stm", name="ps_mean")
                ps_msq = psln.tile([1, S], F32, tag="sts", name="ps_msq")
                for c in range(8):
                    pre_bf = lps.tile([128, S], BF16, tag="prebf", name="prebf")
                    nc.scalar.copy(out=pre_bf, in_=pre_f32[:, c, :])
                    sq_bf = lps.tile([128, S], BF16, tag="sqbf", name="sqbf")
                    nc.scalar.square(out=sq_bf, in_=pre_f32[:, c, :])
                    for h0 in HALVES:
                        nc.tensor.matmul(ps_mean[:, h0:h0 + HALF], lhsT=onesd,
                                         rhs=pre_bf[:, h0:h0 + HALF],
                                         start=(c == 0), stop=(c == 7))
                        nc.tensor.matmul(ps_msq[:, h0:h0 + HALF], lhsT=onesd,
                                         rhs=sq_bf[:, h0:h0 + HALF],
                                         start=(c == 0), stop=(c == 7))
                # matmul rhs operands must sit at base partition 0 (same as
                # the ones lhsT), so mean and invstd live in separate tiles
                lnstat = lst.tile([33, S], F32, tag="lnstat", name="lnstat")
                invt = lst.tile([1, S], F32, tag="invt", name="invt")
                nc.vector.tensor_scalar(out=lnstat[0:1, :], in0=ps_mean,
                                        scalar1=1.0 / 1024.0, scalar2=None, op0=OP.mult)
                nc.vector.tensor_tensor(out=lnstat[32:33, :], in0=lnstat[0:1, :],
                                        in1=lnstat[0:1, :], op=OP.mult)
                nc.vector.scalar_tensor_tensor(out=lnstat[32:33, :], in0=ps_msq,
                                               scalar=1.0 / 1024.0, in1=lnstat[32:33, :],
                                               op0=OP.mult, op1=OP.subtract)
                nc.scalar.activation(out=lnstat[32:33, :], in_=lnstat[32:33, :],
                                     func=AF.Sqrt, bias=eps_ap)
                nc.vector.reciprocal(out=invt[0:1, :], in_=lnstat[32:33, :])

                ps_mb = psln.tile([128, S], F32, tag="bcm", name="ps_mb")
                ps_ib = psln.tile([128, S], F32, tag="bci", name="ps_ib")
                for h0 in HALVES:
                    nc.tensor.matmul(ps_mb[:, h0:h0 + HALF], lhsT=onesr,
                                     rhs=lnstat[0:1, h0:h0 + HALF], start=True, stop=True)
                    nc.tensor.matmul(ps_ib[:, h0:h0 + HALF], lhsT=onesr,
                                     rhs=invt[0:1, h0:h0 + HALF], start=True, stop=True)
                for c in range(8):
                    for h0 in HALVES:
                        t = lps.tile([128, HALF], F32, tag="lnt", name="lnt")
                        nc.vector.tensor_tensor(out=t, in0=pre_f32[:, c, h0:h0 + HALF],
                                                in1=ps_mb[:, h0:h0 + HALF], op=OP.subtract)
                        nc.vector.tensor_tensor(out=t, in0=t,
                                                in1=ps_ib[:, h0:h0 + HALF], op=OP.mult)
                        write_out(c, h0, t, gkey, bkey)

        # ---- LN1 -> xf1 ----
        xf1_f32 = big4.tile([128, 8, S], F32, tag="a4", name="xf1_f32")
        xf1_bf = actp.tile([128, 8, S], BF16, tag="a2", name="xf1_bf")

        def wr_ln1(c, h0, t, gkey, bkey):
            nc.vector.tensor_scalar(out=xf1_f32[:, c, h0:h0 + HALF], in0=t,
                                    scalar1=bb(gkey, c), scalar2=bb(bkey, c),
                                    op0=OP.mult, op1=OP.add)
            nc.scalar.copy(out=xf1_bf[:, c, h0:h0 + HALF],
                           in_=xf1_f32[:, c, h0:h0 + HALF])
        emit_ln(preln1, "n1g", "n1b", wr_ln1, "ln1")

        # ==================================================================
        # Stage C: FFN
        # ==================================================================
        preln2 = big4.tile([128, 8, S], F32, tag="a4", name="preln2")
        hT = [actp.tile([128, 8, S], BF16, tag="a2", name=f"hT{i}") for i in range(3)]
        with tc.tile_pool(name="pc2", bufs=2) as pc2, \
             tc.tile_pool(name="pch", bufs=1) as pch, \
             tc.tile_pool(name="psCmm", bufs=4, space="PSUM") as psmm:
            hT.append(pch.tile([128, 8, S], BF16, tag="hT3", name="hT3"))

            def ev_ffn1(mi, m, h0, ps):
                nc.scalar.activation(out=hT[m // 8][:, m % 8, h0:h0 + HALF], in_=ps,
                                     func=AF.Gelu, bias=bb("b1", m))
            proj(T["w1_blk"], range(32), 8, lambda c, h0, n: xf1_bf[:, c, h0:h0 + n],
                 ev_ffn1, pc2, "wC", psmm, "mm")

            def ev_ffn2(mi, m, h0, ps):
                nc.vector.scalar_tensor_tensor(out=preln2[:, m, h0:h0 + HALF], in0=ps,
                                               scalar=bb("b2", m),
                                               in1=xf1_f32[:, m, h0:h0 + HALF],
                                               op0=OP.add, op1=OP.add)
            proj(T["w2_blk"], range(8), 32,
                 lambda c, h0, n: hT[c // 8][:, c % 8, h0:h0 + n],
                 ev_ffn2, pc2, "wC", psmm, "mm")

        # ---- LN2 -> y (fp32 to DRAM) + yT_bf ----
        yT_bf = actp.tile([128, 8, S], BF16, tag="a2", name="yT_bf")
        with tc.tile_pool(name="ystgp", bufs=2) as ystg_pool:
            ystg = [None] * 8

            def wr_ln2(c, h0, t, gkey, bkey):
                if ystg[c] is None:
                    ystg[c] = ystg_pool.tile([128, S], F32, tag="ystg", name=f"ystg{c}")
                nc.vector.tensor_scalar(out=ystg[c][:, h0:h0 + HALF], in0=t,
                                        scalar1=bb(gkey, c), scalar2=bb(bkey, c),
                                        op0=OP.mult, op1=OP.add)
                if h0 == HALVES[-1]:
                    nc.sync.dma_start(out=T["yT_out"][c * 128:(c + 1) * 128, :],
                                      in_=ystg[c])
                    nc.scalar.copy(out=yT_bf[:, c, :], in_=ystg[c])
            emit_ln(preln2, "n2g", "n2b", wr_ln2, "ln2")
        big4.release()

        # ==================================================================
        # Stage D: pattern pooling MHA + selection head
        # ==================================================================
        with tc.tile_pool(name="pd", bufs=1) as pd, \
             tc.tile_pool(name="pd2", bufs=2) as pd2, \
             tc.tile_pool(name="psDmm", bufs=2, space="PSUM") as psmm, \
             tc.tile_pool(name="psDsc", bufs=3, space="PSUM") as pssc, \
             tc.tile_pool(name="psDo", bufs=2, space="PSUM") as pso_p:

            pkT = actp.tile([128, 8, S], BF16, tag="a2", name="pkT")
            def ev_pk(mi, m, h0, ps):
                nc.vector.tensor_scalar(out=pkT[:, m, h0:h0 + HALF], in0=ps,
                                        scalar1=bb("pkb", m), scalar2=None, op0=OP.add)
            proj(T["pk_blk"], range(8), 8, lambda c, h0, n: yT_bf[:, c, h0:h0 + n],
                 ev_pk, pd2, "wD", psmm, "mm")

            pvaug = pd.tile([128, 8, H * (DH + 1)], BF16, tag="pvaug")
            for hi, h0 in enumerate(HALVES):
                pvh = pd2.tile([128, 8, HALF], BF16, tag="pvh", name=f"pvh{hi}", bufs=1)
                nc.sync.dma_start(out=pvh, in_=T["pv_pcn"][:, :, h0:h0 + HALF])
                for st in range(8):
                    ps = psmm.tile([128, HALF], F32, tag="mm", name="pspv")
                    for c in range(8):
                        nc.tensor.matmul(ps, lhsT=yT_bf[:, c, st * 128:(st + 1) * 128],
                                         rhs=pvh[:, c, :], start=(c == 0), stop=False)
                    nc.tensor.matmul(ps, lhsT=ones1,
                                     rhs=rowb[:, RB["pvb"] + h0:RB["pvb"] + h0 + HALF],
                                     start=False, stop=True)
                    nc.vector.tensor_copy(
                        out=pvaug[:, st, :].rearrange("p (h e) -> p h e", e=DH + 1)
                        [:, hi * 8:(hi + 1) * 8, 0:DH],
                        in_=ps.rearrange("p (h e) -> p h e", e=DH))
            for st in range(8):
                nc.vector.memset(
                    pvaug[:, st, :].rearrange("p (h e) -> p h e", e=DH + 1)[:, :, DH:DH + 1],
                    1.0)

            # packed bf16 bank for the small pattern/selection tensors
            # cols: queries 0:16, pqT 16:32, opat 32:48, patsTbf 48:64,
            #       selw 64:128, sqw 128:384, anw 384, gatew 385
            selbank = pd.tile([128, 8, 386], BF16, tag="selbank")
            nc.sync.dma_start(out=selbank[:, :, 0:16], in_=T["queries_pcn"])
            nc.sync.dma_start(out=selbank[:, :, 64:128], in_=T["selw_pcn"])
            nc.sync.dma_start(out=selbank[:, :, 128:384], in_=T["sqw_pcn"])
            nc.sync.dma_start(out=selbank[:, :, 384:385], in_=T["anw_pcn"])
            nc.sync.dma_start(out=selbank[:, :, 385:386], in_=T["gatew_pcn"])
            queries = selbank[:, :, 0:16]
            pqT = selbank[:, :, 16:32]
            opat = selbank[:, :, 32:48]
            patsT_bf = selbank[:, :, 48:64]
            selw = selbank[:, :, 64:128]
            sqw = selbank[:, :, 128:384]
            anw = selbank[:, :, 384:385]
            gatew = selbank[:, :, 385:386]

            # cols: skw 0:256, slotemb 256:320, skT 320:384, sqT 384:400
            skpack = pd.tile([128, 2, 400], BF16, tag="skpack")
            nc.sync.dma_start(out=skpack[:, :, 0:DC], in_=T["skw_pcn"])
            nc.sync.dma_start(out=skpack[:, :, DC:DC + KSL], in_=T["slotemb_pcn"])
            skw = skpack[:, :, 0:DC]
            slotemb = skpack[:, :, DC:DC + KSL]
            skT = skpack[:, :, 320:384]
            sqT = skpack[:, :, 384:400]

            # pattern q^T
            def ev_pq(mi, m, h0, ps):
                nc.vector.tensor_scalar(out=pqT[:, m, :], in0=ps,
                                        scalar1=bb("pqb", m), scalar2=1.0 / 8.0,
                                        op0=OP.add, op1=OP.mult)
            proj(T["pq_blk"], range(8), 8, lambda c, h0, n: queries[:, c, :],
                 ev_pq, pd2, "wD", psmm, "mm", nfree=PP, nstep=PP)

            # pattern heads
            for h in range(H):
                hp = (h % 2) * 64
                hc = h // 2
                probs = pd2.tile([128, 8, PP], BF16, tag="pprobs", name=f"pprobs{h}")
                for kt in range(8):
                    ps = pssc.tile([128, PP], F32, tag="sd", name="pspsc")
                    nc.tensor.matmul(ps, lhsT=pkT[hp:hp + 64, hc, kt * 128:(kt + 1) * 128],
                                     rhs=pqT[hp:hp + 64, hc, :], start=True, stop=True)
                    nc.scalar.activation(out=probs[:, kt, :], in_=ps, func=AF.Exp)
                pso = pso_p.tile([DH + 1, PP], F32, tag="po", name="pspo")
                for kt in range(8):
                    nc.tensor.matmul(pso, lhsT=pvaug[:, kt, h * (DH + 1):(h + 1) * (DH + 1)],
                                     rhs=probs[:, kt, :], start=(kt == 0), stop=(kt == 7))
                rec = pd2.tile([1, PP], F32, tag="prec", name="prec")
                nc.vector.reciprocal(out=rec, in_=pso[DH:DH + 1, :])
                recb = pd2.tile([64, PP], F32, tag="precb", name="precb")
                nc.gpsimd.partition_broadcast(recb, rec, channels=64)
                nc.vector.tensor_tensor(out=opat[hp:hp + 64, hc, :],
                                        in0=pso[0:DH, :], in1=recb, op=OP.mult)

            # patterns out proj (fp32 -> DRAM) + bf16 copy
            patsT = pd.tile([128, 8, PP], F32, tag="patsT")
            def ev_pwo(mi, m, h0, ps):
                nc.vector.tensor_scalar(out=patsT[:, m, :], in0=ps,
                                        scalar1=bb("pwob", m), scalar2=None, op0=OP.add)
                nc.vector.tensor_copy(out=patsT_bf[:, m, :], in_=patsT[:, m, :])
            proj(T["pwo_blk"], range(8), 8, lambda c, h0, n: opat[:, c, :],
                 ev_pwo, pd2, "wD", psmm, "mm", nfree=PP, nstep=PP)
            nc.sync.dma_start(out=T["patsT_out"].rearrange("c p n -> p c n"), in_=patsT)

            # ---- selection head ----
            # sel2d cols: learned 0:16, tt 16:32, c16 32:48, logits 48:64,
            # lg 64:80, alphabc 80:96, gT 96:112, esb0 112:176, esb1 176:240,
            # stats 240..255
            sel2d = pd.tile([128, 256], F32, tag="sel2d")
            nc.sync.dma_start(out=sel2d[0:KSL, 96:112], in_=T["gT"])
            ident = pd.tile([128, 128], F32, tag="ident")
            make_identity(nc, ident)
            sc_out = pd.tile([1, 2 * PP], F32, tag="sc_out")

            ps_s = pssc.tile([1, PP], F32, tag="sd", name="ps_s")
            for c in range(8):
                nc.tensor.matmul(ps_s, lhsT=gatew[:, c, :], rhs=patsT_bf[:, c, :],
                                 start=(c == 0), stop=(c == 7))
            nc.scalar.activation(out=sc_out[:, 0:PP], in_=ps_s, func=AF.Sigmoid,
                                 bias=bank[0:1, BB["gateb"]:BB["gateb"] + 1])
            nc.sync.dma_start(out=T["scores_out"], in_=sc_out[:, 0:PP])

            ps_a = pssc.tile([1, PP], F32, tag="sd", name="ps_a")
            for c in range(8):
                nc.tensor.matmul(ps_a, lhsT=anw[:, c, :], rhs=patsT_bf[:, c, :],
                                 start=(c == 0), stop=(c == 7))
            nc.scalar.activation(out=sc_out[:, PP:2 * PP], in_=ps_a, func=AF.Sigmoid,
                                 bias=bank[0:1, BB["anb"]:BB["anb"] + 1])
            nc.sync.dma_start(out=T["alpha_out"], in_=sc_out[:, PP:2 * PP])
            alphabc = sel2d[0:KSL, 80:96]
            nc.gpsimd.partition_broadcast(alphabc, sc_out[:, PP:2 * PP], channels=KSL)

            ps_l = pso_p.tile([KSL, PP], F32, tag="po", name="ps_l")
            for c in range(8):
                nc.tensor.matmul(ps_l, lhsT=selw[:, c, :], rhs=patsT_bf[:, c, :],
                                 start=(c == 0), stop=(c == 7))
            learned = sel2d[0:KSL, 0:16]
            nc.vector.tensor_scalar(out=learned, in0=ps_l,
                                    scalar1=bank[0:KSL, BB["selb"]:BB["selb"] + 1],
                                    scalar2=None, op0=OP.add)
            for m in range(2):
                ps = pssc.tile([128, PP], F32, tag="sd", name="ps_sq")
                for c in range(8):
                    nc.tensor.matmul(ps, lhsT=sqw[:, c, m * 128:(m + 1) * 128],
                                     rhs=patsT_bf[:, c, :], start=(c == 0), stop=(c == 7))
                nc.vector.tensor_scalar(out=sqT[:, m, :], in0=ps,
                                        scalar1=bb("sqb", m), scalar2=None, op0=OP.add)
            for m in range(2):
                ps = pssc.tile([128, KSL], F32, tag="sd", name="ps_sk")
                for c in range(2):
                    nc.tensor.matmul(ps, lhsT=skw[:, c, m * 128:(m + 1) * 128],
                                     rhs=slotemb[:, c, :], start=(c == 0), stop=(c == 1))
                nc.vector.tensor_scalar(out=skT[:, m, :], in0=ps,
                                        scalar1=bb("skb", m), scalar2=None, op0=OP.add)
            ps_c = pso_p.tile([KSL, PP], F32, tag="po", name="ps_c")
            for c in range(2):
                nc.tensor.matmul(ps_c, lhsT=skT[:, c, :], rhs=sqT[:, c, :],
                                 start=(c == 0), stop=(c == 1))
            tt = sel2d[0:KSL, 16:32]
            nc.vector.scalar_tensor_tensor(out=tt, in0=ps_c, scalar=-1.0 / 16.0,
                                           in1=learned, op0=OP.mult, op1=OP.add)
            c16 = sel2d[0:KSL, 32:48]
            nc.vector.tensor_scalar(out=c16, in0=ps_c, scalar1=1.0 / 16.0,
                                    scalar2=None, op0=OP.mult)
            logits = sel2d[0:KSL, 48:64]
            nc.vector.tensor_tensor(out=logits, in0=tt, in1=alphabc, op=OP.mult)
            nc.vector.tensor_tensor(out=logits, in0=logits, in1=c16, op=OP.add)
            lg = sel2d[0:KSL, 64:80]
            nc.vector.tensor_tensor(out=lg, in0=logits, in1=sel2d[0:KSL, 96:112],
                                    op=OP.add)

            for idx, (src, out_name) in enumerate(((lg, "slotp_out"),
                                                   (logits, "softp_out"))):
                ps_t = pssc.tile([PP, KSL], F32, tag="sd", name=f"ps_t{idx}")
                nc.tensor.transpose(ps_t, src, ident[0:KSL, 0:KSL])
                mx = sel2d[0:PP, 240 + idx * 8:241 + idx * 8]
                nc.vector.tensor_reduce(out=mx, in_=ps_t, axis=mybir.AxisListType.X,
                                        op=OP.max)
                mxn = sel2d[0:PP, 241 + idx * 8:242 + idx * 8]
                nc.vector.tensor_scalar(out=mxn, in0=mx, scalar1=-1.0, scalar2=None,
                                        op0=OP.mult)
                esb = sel2d[0:PP, 112 + idx * 64:176 + idx * 64]
                ssum = sel2d[0:PP, 242 + idx * 8:243 + idx * 8]
                nc.scalar.activation(out=esb, in_=ps_t, func=AF.Exp, bias=mxn,
                                     accum_out=ssum)
                rs = sel2d[0:PP, 243 + idx * 8:244 + idx * 8]
                nc.vector.reciprocal(out=rs, in_=ssum)
                osb = pd2.tile([PP, KSL], F32, tag="osb", name=f"osb{idx}")
                nc.vector.tensor_scalar(out=osb, in0=esb, scalar1=rs, scalar2=None,
                                        op0=OP.mult)
                nc.sync.dma_start(out=T[out_name], in_=osb)


def build_nc():
    nc = bacc.Bacc("TRN2", target_bir_lowering=False, debug=False)
    T = _declare_tensors(nc)
    with tile.TileContext(nc) as tc:
        _emit(nc, tc, T)
    nc.compile()
    return nc


# ----------------------------------------------------------------------------
# host side
# ----------------------------------------------------------------------------

def host_prep(inputs):
    f32 = np.float32
    inp = {k: np.asarray(v) for k, v in inputs.items()}

    wqkvT = inp["attn_wqkv"].T.astype(f32)
    pwT = inp["pat_wqkv"].T.astype(f32)
    decw = inp["dec_w"].astype(f32)
    decb_eff = inp["dec_b"].astype(f32) + \
        inp["layer_embed"].reshape(-1).astype(f32) @ decw[:, DC:].T

    bankv = np.zeros((128, BBW), f32)
    def setb(key, arr):
        t = _bias_t(arr)
        bankv[:, BB[key]:BB[key] + t.shape[1]] = t
    setb("cqb", inp["cq_b"]); setb("ckb", inp["ck_b"]); setb("decb", decb_eff)
    setb("fgb", inp["fg_b"]); setb("bqk", inp["attn_bqkv"][:2 * D])
    setb("wob", inp["attn_bo"]); setb("n1g", inp["n1_g"]); setb("n1b", inp["n1_b"])
    setb("n2g", inp["n2_g"]); setb("n2b", inp["n2_b"])
    setb("b1", inp["ffn_b1"]); setb("b2", inp["ffn_b2"])
    setb("pqb", inp["pat_bqkv"][:D]); setb("pkb", inp["pat_bqkv"][D:2 * D])
    setb("pwob", inp["pat_bo"]); setb("sqb", inp["sq_b"]); setb("skb", inp["sk_b"])
    bankv[0:KSL, BB["selb"]] = inp["sel_b"].astype(f32)
    bankv[0, BB["anb"]] = np.float32(inp["an_b"][0])
    bankv[0, BB["gateb"]] = np.float32(inp["gate_b"][0])
    bankv[0, BB["eps"]] = 1e-5
    bankv[:, BB["onesr"]:BB["onesr"] + 128] = 1.0

    rowv = np.zeros((1, 2304), BFNP)
    rowv[0, RB["cvb"]:RB["cvb"] + DC] = inp["cv_b"].astype(BFNP)
    rowv[0, RB["bv"]:RB["bv"] + D] = inp["attn_bqkv"][2 * D:].astype(BFNP)
    rowv[0, RB["pvb"]:RB["pvb"] + D] = inp["pat_bqkv"][2 * D:].astype(BFNP)

    shared = {
        "biasbank": bankv,
        "rowbank": rowv,
        "cqw_blk": _blk(inp["cq_w"].T),
        "ckw_pcn": _pcn(_pad_rows(inp["ck_w"].T.astype(f32), 384)),
        "cvw_pcn": _pcn(_pad_rows(inp["cv_w"].T.astype(f32), 384)),
        "decw_blk": _blk(decw[:, :DC].T),
        "fgw_blk": _blk(inp["fg_w"].T),
        "wqk_blk": _blk(wqkvT[:, :2 * D]),
        "wv_pcn": _pcn(wqkvT[:, 2 * D:]),
        "wo_blk": _blk(inp["attn_wo"].T),
        "w1_blk": _blk(inp["ffn_w1"].T),
        "w2_blk": _blk(inp["ffn_w2"].T),
        "pq_blk": _blk(pwT[:, :D]),
        "pk_blk": _blk(pwT[:, D:2 * D]),
        "pv_pcn": _pcn(pwT[:, 2 * D:]),
        "pwo_blk": _blk(inp["pat_wo"].T),
        "queries_pcn": _pcn(inp["pattern_queries"].T.astype(f32)),
        "selw_pcn": _pcn(inp["sel_w"].T.astype(f32)),
        "sqw_pcn": _pcn(inp["sq_w"].T.astype(f32)),
        "skw_pcn": _pcn(inp["sk_w"].T.astype(f32)),
        "slotemb_pcn": _pcn(inp["slot_embeddings"].T.astype(f32)),
        "anw_pcn": _pcn(inp["an_w"].T.astype(f32)),
        "gatew_pcn": _pcn(inp["gate_w"].T.astype(f32)),
    }

    g_all = -np.log(-np.log(inp["gumbel_u"].astype(f32) + 1e-8) + 1e-8)
    lids_T = inp["layer_ids"].T.astype(f32)

    in_maps = []
    for b in range(NCORES):
        cidT = np.concatenate([inp["cache"][b].T.astype(f32), lids_T], axis=0)
        m = dict(shared)
        m["xT"] = np.ascontiguousarray(inp["x"][b].T.astype(f32))
        m["cid_pcn"] = _pcn(_pad_rows(cidT, 384))
        m["gT"] = np.ascontiguousarray(g_all[b].T)
        in_maps.append(m)
    return in_maps


def _get_nc():
    if "nc" not in _CACHE:
        _CACHE["nc"] = build_nc()
    return _CACHE["nc"]


def run_on_hw(in_maps, **kw):
    nc = _get_nc()
    return bass_utils.run_bass_kernel_spmd(nc, in_maps, core_ids=list(range(NCORES)), **kw)


def assemble_outputs(results):
    y = np.stack([r["yT_out"].T for r in results])
    patterns = np.stack([r["patsT_out"].reshape(D, PP).T for r in results])
    scores = np.stack([r["scores_out"][0] for r in results])
    slot_probs = np.stack([r["slotp_out"] for r in results])
    soft_probs = np.stack([r["softp_out"] for r in results])
    alpha = np.stack([r["alpha_out"][0] for r in results])
    return (y, patterns, scores, slot_probs, soft_probs, alpha)


def kernel(**inputs):
    in_maps = host_prep(inputs)
    res = run_on_hw(in_maps)
    return assemble_outputs(res.results)


# revision 16
# speedup vs baseline: 1.0300x; 1.0300x over previous
# Trainium2 Bass kernel for nn_DLSMNLayer (DLSMN layer: cache cross-attention,
# gated fusion, transformer block, pattern pooling, selection head).
#
# Strategy: data-parallel over batch B=8 across the 8 NeuronCores (one batch
# element per core). Inside each core the whole layer runs fused in SBUF:
#   - activations are kept feature-major ("transposed", [feature, token]) so
#     every linear layer is a plain PE matmul with host-pretransposed weights
#   - matmuls run in bf16 with fp32 PSUM accumulation; the residual stream
#     (xf, preln1, xf1, preln2) and both layernorm applications stay fp32
#   - attention is computed transpose-free: scores are produced as S^T
#     [keys, queries], softmax denominators come from an extra ones-column in
#     the value matrix (one PE accumulation), normalization folds into the
#     PSUM eviction
#   - per-token stats (layernorm mean/var, softmax sums) that live across the
#     partition axis are computed with ones-vector PE matmuls and broadcast
#     back across partitions with k=1 PE matmuls / gpsimd partition_broadcast
#   - SBUF tiles pad to 4KB/partition, so small constants are packed into
#     shared "bank" tiles and pools are scoped tightly to stages
#
# kernel(**inputs) takes the full unsharded inputs and returns the same
# 6-tuple as the reference: (y, patterns, scores, slot_probs, soft_probs, alpha)

import contextlib

import numpy as np
import ml_dtypes

import concourse.bacc as bacc
import concourse.bass as bass
import concourse.tile as tile
import concourse.mybir as mybir
from concourse import bass_utils
from concourse.masks import make_identity

F32 = mybir.dt.float32
BF16 = mybir.dt.bfloat16
AF = mybir.ActivationFunctionType
OP = mybir.AluOpType
BFNP = ml_dtypes.bfloat16

D = 1024      # d_model
DC = 256      # d_cache
DL = 64       # d_layer
KSL = 64      # num_slots
NCA = 512     # cache slots
PP = 16       # num_patterns
H = 16        # num_heads
DH = 64       # head dim
B, S = 8, 1024
FF = 4096
NCORES = 8
HALF = 512
HALVES = (0, HALF)

_CACHE = {}

# bias bank column map (f32 [128, 280])
BB = dict(cqb=0, ckb=2, decb=4, fgb=12, bqk=20, wob=36, n1g=44, n1b=52,
          n2g=60, n2b=68, b1=76, b2=108, pqb=116, pkb=124, pwob=132,
          sqb=140, skb=142, selb=144, anb=145, gateb=146, eps=147, onesr=152)
BBW = 280
# bf16 row bank column map ([1, 2304])
RB = dict(cvb=0, bv=256, pvb=1280)


# ----------------------------------------------------------------------------
# host-side weight layout helpers
# ----------------------------------------------------------------------------

def _pcn(wT, dtype=BFNP):
    """[K, M] -> [128, K//128, M]: partition p, chunk c <-> row c*128+p."""
    K, M = wT.shape
    assert K % 128 == 0
    return np.ascontiguousarray(
        np.asarray(wT).reshape(K // 128, 128, M).transpose(1, 0, 2).astype(dtype))


def _blk(wT, dtype=BFNP):
    """[K, M] -> [M//128, 128, K//128, 128] contiguous m-blocks in pcn layout."""
    K, M = wT.shape
    assert K % 128 == 0 and M % 128 == 0
    return np.ascontiguousarray(
        np.asarray(wT).reshape(K // 128, 128, M // 128, 128)
        .transpose(2, 1, 0, 3).astype(dtype))


def _bias_t(b):
    """[L] -> [128, L//128]: column m holds bias for features m*128..m*128+127."""
    L = b.shape[0]
    return np.ascontiguousarray(np.asarray(b, np.float32).reshape(L // 128, 128).T)


def _pad_rows(a, K):
    out = np.zeros((K, a.shape[1]), a.dtype)
    out[: a.shape[0]] = a
    return out


# ----------------------------------------------------------------------------
# device kernel
# ----------------------------------------------------------------------------

def _declare_tensors(nc):
    T = {}

    def di(name, shape, dt):
        T[name] = nc.dram_tensor(name, list(shape), dt, kind="ExternalInput").ap()

    def do(name, shape, dt):
        T[name] = nc.dram_tensor(name, list(shape), dt, kind="ExternalOutput").ap()

    di("xT", (D, S), F32)
    di("cid_pcn", (128, 3, NCA), BF16)
    di("gT", (KSL, PP), F32)

    di("biasbank", (128, BBW), F32)
    di("rowbank", (1, 2304), BF16)

    di("cqw_blk", (2, 128, 8, 128), BF16)
    di("ckw_pcn", (128, 3, DC), BF16)
    di("cvw_pcn", (128, 3, DC), BF16)
    di("decw_blk", (8, 128, 2, 128), BF16)
    di("fgw_blk", (8, 128, 16, 128), BF16)
    di("wqk_blk", (16, 128, 8, 128), BF16)
    di("wv_pcn", (128, 8, D), BF16)
    di("wo_blk", (8, 128, 8, 128), BF16)
    di("w1_blk", (32, 128, 8, 128), BF16)
    di("w2_blk", (8, 128, 32, 128), BF16)
    di("pq_blk", (8, 128, 8, 128), BF16)
    di("pk_blk", (8, 128, 8, 128), BF16)
    di("pv_pcn", (128, 8, D), BF16)
    di("pwo_blk", (8, 128, 8, 128), BF16)
    di("queries_pcn", (128, 8, PP), BF16)
    di("selw_pcn", (128, 8, KSL), BF16)
    di("sqw_pcn", (128, 8, DC), BF16)
    di("skw_pcn", (128, 2, DC), BF16)
    di("slotemb_pcn", (128, 2, KSL), BF16)
    di("anw_pcn", (128, 8, 1), BF16)
    di("gatew_pcn", (128, 8, 1), BF16)

    do("yT_out", (D, S), F32)
    do("patsT_out", (8, 128, PP), F32)
    do("scores_out", (1, PP), F32)
    do("alpha_out", (1, PP), F32)
    do("slotp_out", (PP, KSL), F32)
    do("softp_out", (PP, KSL), F32)
    return T


def _emit(nc, tc, T):
    with contextlib.ExitStack() as ctx:
        # ---------------- constants (packed banks) ----------------
        const = ctx.enter_context(tc.tile_pool(name="const", bufs=1))
        onesall = const.tile([128, 128], BF16, tag="onesall")
        nc.vector.memset(onesall, 1.0)
        ones1 = onesall[0:1, :]          # [1,128] row of ones (bias-row matmuls)
        onesd = onesall[:, 0:1]          # [128,1] column of ones (LN sums)
        bank = const.tile([128, BBW], F32, tag="bank")
        nc.sync.dma_start(out=bank, in_=T["biasbank"])
        rowb = const.tile([1, 2304], BF16, tag="rowb")
        nc.sync.dma_start(out=rowb, in_=T["rowbank"])

        def bb(key, m):
            return bank[:, BB[key] + m:BB[key] + m + 1]

        onesr = bank[0:1, BB["onesr"]:BB["onesr"] + 128]   # [1,128] f32 ones
        eps_ap = bank[0:1, BB["eps"]:BB["eps"] + 1]

        # ---------------- long-lived pools ----------------
        actp = ctx.enter_context(tc.tile_pool(name="actp", bufs=4))    # 16KB tiles
        big4 = tc.alloc_tile_pool(name="big4", bufs=2)                 # 32KB f32 tiles

        # generic blocked projection
        def proj(w_blk_ap, ms, kc, rhs_sel, evict, wpool, wtag, pspool, pstag,
                 nfree=S, nstep=HALF, extra_mm=None):
            for mi, m in enumerate(ms):
                wt = wpool.tile([128, kc, 128], BF16, tag=wtag, name=f"w{wtag}{m}")
                nc.sync.dma_start(out=wt, in_=w_blk_ap[m])
                for h0 in range(0, nfree, nstep):
                    ps = pspool.tile([128, nstep], F32, tag=pstag, name=f"ps{wtag}")
                    for c in range(kc):
                        nc.tensor.matmul(ps, lhsT=wt[:, c, :],
                                         rhs=rhs_sel(c, h0, nstep),
                                         start=(c == 0),
                                         stop=(extra_mm is None and c == kc - 1))
                    if extra_mm is not None:
                        extra_mm(ps, h0, nstep)
                    evict(mi, m, h0, ps)

        xf_f32 = big4.tile([128, 8, S], F32, tag="a4", name="xf_f32")
        xf_bf = actp.tile([128, 8, S], BF16, tag="a2", name="xf_bf")

        # ==================================================================
        # Stage A: read_cache cross-attention + decode + gated fusion
        # ==================================================================
        with tc.tile_pool(name="pa", bufs=1) as pa, \
             tc.tile_pool(name="pa2", bufs=2) as pa2, \
             tc.tile_pool(name="psAmm", bufs=2, space="PSUM") as psmm:

            xT_bf = pa.tile([128, 8, S], BF16, tag="xT_bf")
            for c in range(8):
                xs = pa2.tile([128, S], F32, tag="xld", name=f"xld{c}")
                nc.sync.dma_start(out=xs, in_=T["xT"][c * 128:(c + 1) * 128, :])
                nc.scalar.copy(out=xT_bf[:, c, :], in_=xs)
            ctxT = pa.tile([128, 2, S], BF16, tag="ctxT")

            ca = tc.alloc_tile_pool(name="ca", bufs=1)
            psvv = tc.alloc_tile_pool(name="psAvv", bufs=2, space="PSUM")
            pssum = tc.alloc_tile_pool(name="psAsum", bufs=2, space="PSUM")
            psctx = tc.alloc_tile_pool(name="psActx", bufs=2, space="PSUM")

            cid = ca.tile([128, 3, NCA], BF16, tag="cid")
            nc.sync.dma_start(out=cid, in_=T["cid_pcn"])
            ckcv = ca.tile([128, 3, 2 * DC], BF16, tag="ckcv")
            nc.sync.dma_start(out=ckcv[:, :, 0:DC], in_=T["ckw_pcn"])
            nc.sync.dma_start(out=ckcv[:, :, DC:2 * DC], in_=T["cvw_pcn"])

            # q_ca = (x @ cq_w.T + cq_b)/16
            qca = ca.tile([128, 2, S], BF16, tag="qca")
            def ev_qca(mi, m, h0, ps):
                nc.vector.tensor_scalar(out=qca[:, m, h0:h0 + HALF], in0=ps,
                                        scalar1=bb("cqb", m), scalar2=1.0 / 16.0,
                                        op0=OP.add, op1=OP.mult)
            proj(T["cqw_blk"], range(2), 8, lambda c, h0, n: xT_bf[:, c, h0:h0 + n],
                 ev_qca, pa2, "wA", psmm, "mm")

            # kk^T [dc, n]
            kkT = ca.tile([128, 2, NCA], BF16, tag="kkT")
            for m in range(2):
                ps = psmm.tile([128, NCA], F32, tag="mm", name=f"pskk{m}")
                for c in range(3):
                    nc.tensor.matmul(ps, lhsT=ckcv[:, c, m * 128:(m + 1) * 128],
                                     rhs=cid[:, c, :], start=(c == 0), stop=(c == 2))
                nc.vector.tensor_scalar(out=kkT[:, m, :], in0=ps,
                                        scalar1=bb("ckb", m), scalar2=None, op0=OP.add)

            # vv natural [n, dc] + ones column
            vvaug = ca.tile([128, 4, DC + 1], BF16, tag="vvaug")
            for nt in range(4):
                ps = psvv.tile([128, DC], F32, tag="vv", name=f"psvv{nt}")
                for c in range(3):
                    nc.tensor.matmul(ps, lhsT=cid[:, c, nt * 128:(nt + 1) * 128],
                                     rhs=ckcv[:, c, DC:2 * DC], start=(c == 0), stop=False)
                nc.tensor.matmul(ps, lhsT=ones1, rhs=rowb[:, RB["cvb"]:RB["cvb"] + DC],
                                 start=False, stop=True)
                nc.vector.tensor_copy(out=vvaug[:, nt, 0:DC], in_=ps)
                nc.vector.memset(vvaug[:, nt, DC:DC + 1], 1.0)

            # probs^T = exp(kk q / 16)
            pca = ca.tile([128, 4, S], BF16, tag="pca")
            for nt in range(4):
                for h0 in HALVES:
                    ps = psmm.tile([128, HALF], F32, tag="mm", name=f"pssc{nt}")
                    for c in range(2):
                        nc.tensor.matmul(ps, lhsT=kkT[:, c, nt * 128:(nt + 1) * 128],
                                         rhs=qca[:, c, h0:h0 + HALF],
                                         start=(c == 0), stop=(c == 1))
                    nc.scalar.activation(out=pca[:, nt, h0:h0 + HALF], in_=ps, func=AF.Exp)

            # ctx^T [dc, s] with ones-column sums
            recip_ca = ca.tile([1, S], F32, tag="recip_ca")
            for h0 in HALVES:
                pss = pssum.tile([1, HALF], F32, tag="casum", name="pscas")
                for nt in range(4):
                    nc.tensor.matmul(pss, lhsT=vvaug[:, nt, DC:DC + 1],
                                     rhs=pca[:, nt, h0:h0 + HALF],
                                     start=(nt == 0), stop=(nt == 3))
                nc.vector.reciprocal(out=recip_ca[:, h0:h0 + HALF], in_=pss)
            recb_ca = ca.tile([128, S], F32, tag="recb_ca")
            nc.gpsimd.partition_broadcast(recb_ca, recip_ca, channels=128)

            for m in range(2):
                for h0 in HALVES:
                    ps = psctx.tile([128, HALF], F32, tag="ctx", name="psctx")
                    for nt in range(4):
                        nc.tensor.matmul(ps, lhsT=vvaug[:, nt, m * 128:(m + 1) * 128],
                                         rhs=pca[:, nt, h0:h0 + HALF],
                                         start=(nt == 0), stop=(nt == 3))
                    nc.vector.tensor_tensor(out=ctxT[:, m, h0:h0 + HALF], in0=ps,
                                            in1=recb_ca[:, h0:h0 + HALF], op=OP.mult)
            ca.release()
            psctx.release()
            pssum.release()
            psvv.release()

            # context^T = dec_w[:, :256] @ ctx^T + decb_eff (layer_embed folded)
            context_bf = actp.tile([128, 8, S], BF16, tag="a2", name="context_bf")
            def ev_dec(mi, m, h0, ps):
                nc.vector.tensor_scalar(out=context_bf[:, m, h0:h0 + HALF], in0=ps,
                                        scalar1=bb("decb", m), scalar2=None, op0=OP.add)
            proj(T["decw_blk"], range(8), 2, lambda c, h0, n: ctxT[:, c, h0:h0 + n],
                 ev_dec, pa2, "wA", psmm, "mm")

            # gated fusion -> xf (fp32) + xf_bf
            pg = tc.alloc_tile_pool(name="pg", bufs=2)
            def fg_rhs(c, h0, n):
                return (xT_bf if c < 8 else context_bf)[:, c % 8, h0:h0 + n]
            def ev_fg(mi, m, h0, ps):
                g = pg.tile([128, HALF], F32, tag="gsig", name="gsig")
                nc.scalar.activation(out=g, in_=ps, func=AF.Sigmoid, bias=bb("fgb", m))
                dd = pg.tile([128, HALF], F32, tag="gd", name="gd")
                nc.vector.tensor_tensor(out=dd, in0=xT_bf[:, m, h0:h0 + HALF],
                                        in1=context_bf[:, m, h0:h0 + HALF], op=OP.subtract)
                nc.vector.tensor_tensor(out=dd, in0=dd, in1=g, op=OP.mult)
                nc.vector.tensor_tensor(out=xf_f32[:, m, h0:h0 + HALF], in0=dd,
                                        in1=context_bf[:, m, h0:h0 + HALF], op=OP.add)
                nc.scalar.copy(out=xf_bf[:, m, h0:h0 + HALF],
                               in_=xf_f32[:, m, h0:h0 + HALF])
            proj(T["fgw_blk"], range(8), 16, fg_rhs, ev_fg, pa2, "wA", psmm, "mm")
            pg.release()

        # ==================================================================
        # Stage B: self-attention block
        # ==================================================================
        preln1 = big4.tile([128, 8, S], F32, tag="a4", name="preln1")
        o_bf = actp.tile([128, 8, S], BF16, tag="a2", name="o_bf")
        with tc.tile_pool(name="pb", bufs=1) as pb, \
             tc.tile_pool(name="pb2", bufs=2) as pb2, \
             tc.tile_pool(name="probsp", bufs=6) as probsp, \
             tc.tile_pool(name="psBmm", bufs=2, space="PSUM") as psmm, \
             tc.tile_pool(name="psBsc", bufs=3, space="PSUM") as pssc, \
             tc.tile_pool(name="psBo", bufs=3, space="PSUM") as pso_p:

            # v natural with per-head ones columns (half-major so wv streams once)
            vaug = pb.tile([128, 8, H * (DH + 1)], BF16, tag="vaug")
            for hi, h0 in enumerate(HALVES):
                wvh = pb2.tile([128, 8, HALF], BF16, tag="wvh", name=f"wvh{hi}", bufs=1)
                nc.sync.dma_start(out=wvh, in_=T["wv_pcn"][:, :, h0:h0 + HALF])
                for st in range(8):
                    ps = psmm.tile([128, HALF], F32, tag="mm", name="psv")
                    for c in range(8):
                        nc.tensor.matmul(ps, lhsT=xf_bf[:, c, st * 128:(st + 1) * 128],
                                         rhs=wvh[:, c, :], start=(c == 0), stop=False)
                    nc.tensor.matmul(ps, lhsT=ones1,
                                     rhs=rowb[:, RB["bv"] + h0:RB["bv"] + h0 + HALF],
                                     start=False, stop=True)
                    nc.vector.tensor_copy(
                        out=vaug[:, st, :].rearrange("p (h e) -> p h e", e=DH + 1)
                        [:, hi * 8:(hi + 1) * 8, 0:DH],
                        in_=ps.rearrange("p (h e) -> p h e", e=DH))
            for st in range(8):
                nc.vector.memset(
                    vaug[:, st, :].rearrange("p (h e) -> p h e", e=DH + 1)[:, :, DH:DH + 1],
                    1.0)

            # q/k head groups of 4 heads (2 feature chunks each)
            for grp in range(4):
                qg = pb2.tile([128, 2, S], BF16, tag="qg", name=f"qg{grp}")
                kg = pb2.tile([128, 2, S], BF16, tag="kg", name=f"kg{grp}")
                def ev_qk(mi, m, h0, ps, qg=qg, kg=kg):
                    if m < 8:
                        nc.vector.tensor_scalar(out=qg[:, mi, h0:h0 + HALF], in0=ps,
                                                scalar1=bb("bqk", m), scalar2=1.0 / 8.0,
                                                op0=OP.add, op1=OP.mult)
                    else:
                        nc.vector.tensor_scalar(out=kg[:, mi - 2, h0:h0 + HALF], in0=ps,
                                                scalar1=bb("bqk", m), scalar2=None,
                                                op0=OP.add)
                proj(T["wqk_blk"], [2 * grp, 2 * grp + 1, 8 + 2 * grp, 9 + 2 * grp], 8,
                     lambda c, h0, n: xf_bf[:, c, h0:h0 + n],
                     ev_qk, pb2, "wB", psmm, "mm")

                for hh in range(4):
                    h = grp * 4 + hh
                    hp = (hh % 2) * 64
                    hc = hh // 2
                    for h0 in HALVES:
                        pso = pso_p.tile([DH + 1, HALF], F32, tag="o", name="pso")
                        for kt in range(8):
                            ps = pssc.tile([128, HALF], F32, tag="sc", name="pssc")
                            nc.tensor.matmul(
                                ps, lhsT=kg[hp:hp + 64, hc, kt * 128:(kt + 1) * 128],
                                rhs=qg[hp:hp + 64, hc, h0:h0 + HALF],
                                start=True, stop=True)
                            probs = probsp.tile([128, HALF], BF16, tag="probs",
                                                name=f"probs{h}_{kt}")
                            nc.scalar.activation(out=probs, in_=ps, func=AF.Exp)
                            nc.tensor.matmul(pso,
                                             lhsT=vaug[:, kt, h * (DH + 1):(h + 1) * (DH + 1)],
                                             rhs=probs,
                                             start=(kt == 0), stop=(kt == 7))
                        rec = pb2.tile([1, HALF], F32, tag="rec", name="rec")
                        nc.vector.reciprocal(out=rec, in_=pso[DH:DH + 1, :])
                        recb = pb2.tile([64, HALF], F32, tag="recb", name="recb")
                        nc.gpsimd.partition_broadcast(recb, rec, channels=64)
                        nc.vector.tensor_tensor(
                            out=o_bf[(h % 2) * 64:(h % 2) * 64 + 64, h // 2, h0:h0 + HALF],
                            in0=pso[0:DH, :], in1=recb, op=OP.mult)

            # out proj + residual -> preln1 (fp32)
            def ev_wo(mi, m, h0, ps):
                nc.vector.scalar_tensor_tensor(out=preln1[:, m, h0:h0 + HALF], in0=ps,
                                               scalar=bb("wob", m),
                                               in1=xf_f32[:, m, h0:h0 + HALF],
                                               op0=OP.add, op1=OP.add)
            proj(T["wo_blk"], range(8), 8, lambda c, h0, n: o_bf[:, c, h0:h0 + n],
                 ev_wo, pb2, "wB", psmm, "mm")

        # ==================================================================
        # layernorm (feature axis = partitions, PE-based stats)
        # ==================================================================
        onescol_f = bank[:, BB["onesr"]:BB["onesr"] + 1]   # [128,1] f32 ones

        def emit_ln(pre_f32, gkey, bkey, write_out, lnname):
            with tc.tile_pool(name=f"{lnname}s", bufs=2) as lps, \
                 tc.tile_pool(name=f"{lnname}st", bufs=1) as lst, \
                 tc.tile_pool(name=f"{lnname}ps", bufs=1, space="PSUM") as psln:
                # sums via PE: mean from fp32 preln directly, squares via ACT
                ps_mean = psln.tile([1, S], F32, tag="stm", name="ps_mean")
                ps_msq = psln.tile([1, S], F32, tag="sts", name="ps_msq")
                for c in range(8):
                    sq_bf = lps.tile([128, S], BF16, tag="sqbf", name="sqbf")
                    nc.scalar.square(out=sq_bf, in_=pre_f32[:, c, :])
                    for h0 in HALVES:
                        nc.tensor.matmul(ps_mean[:, h0:h0 + HALF], lhsT=onescol_f,
                                         rhs=pre_f32[:, c, h0:h0 + HALF],
                                         start=(c == 0), stop=(c == 7))
                        nc.tensor.matmul(ps_msq[:, h0:h0 + HALF], lhsT=onesd,
                                         rhs=sq_bf[:, h0:h0 + HALF],
                                         start=(c == 0), stop=(c == 7))
                lnstat = lst.tile([33, S], F32, tag="lnstat", name="lnstat")
                invt = lst.tile([1, S], F32, tag="invt", name="invt")
                nc.vector.tensor_scalar(out=lnstat[0:1, :], in0=ps_mean,
                                        scalar1=1.0 / 1024.0, scalar2=None, op0=OP.mult)
                nc.vector.tensor_tensor(out=lnstat[32:33, :], in0=lnstat[0:1, :],
                                        in1=lnstat[0:1, :], op=OP.mult)
                nc.vector.scalar_tensor_tensor(out=lnstat[32:33, :], in0=ps_msq,
                                               scalar=1.0 / 1024.0, in1=lnstat[32:33, :],
                                               op0=OP.mult, op1=OP.subtract)
                nc.scalar.activation(out=lnstat[32:33, :], in_=lnstat[32:33, :],
                                     func=AF.Sqrt, bias=eps_ap)
                nc.vector.reciprocal(out=invt[0:1, :], in_=lnstat[32:33, :])

                # broadcast per-token stats across partitions on gpsimd (SBUF,
                # no PSUM banks, no pool-release barrier for the next stage)
                mb_sb = lst.tile([128, S], F32, tag="mb_sb", name="mb_sb")
                ib_sb = lst.tile([128, S], F32, tag="ib_sb", name="ib_sb")
                nc.gpsimd.partition_broadcast(mb_sb, lnstat[0:1, :], channels=128)
                nc.gpsimd.partition_broadcast(ib_sb, invt[0:1, :], channels=128)
                for c in range(8):
                    for h0 in HALVES:
                        t = lps.tile([128, HALF], F32, tag="lnt", name="lnt")
                        nc.vector.tensor_tensor(out=t, in0=pre_f32[:, c, h0:h0 + HALF],
                                                in1=mb_sb[:, h0:h0 + HALF], op=OP.subtract)
                        nc.vector.tensor_tensor(out=t, in0=t,
                                                in1=ib_sb[:, h0:h0 + HALF], op=OP.mult)
                        write_out(c, h0, t, gkey, bkey)

        # ---- LN1 -> xf1 ----
        pc2 = tc.alloc_tile_pool(name="pc2", bufs=2)
        psC = tc.alloc_tile_pool(name="psCmm", bufs=4, space="PSUM")
        xf1_f32 = big4.tile([128, 8, S], F32, tag="a4", name="xf1_f32")
        xf1_bf = actp.tile([128, 8, S], BF16, tag="a2", name="xf1_bf")

        def wr_ln1(c, h0, t, gkey, bkey):
            nc.vector.tensor_scalar(out=xf1_f32[:, c, h0:h0 + HALF], in0=t,
                                    scalar1=bb(gkey, c), scalar2=bb(bkey, c),
                                    op0=OP.mult, op1=OP.add)
            nc.scalar.copy(out=xf1_bf[:, c, h0:h0 + HALF],
                           in_=xf1_f32[:, c, h0:h0 + HALF])
        emit_ln(preln1, "n1g", "n1b", wr_ln1, "ln1")

        # ==================================================================
        # Stage C: FFN
        # ==================================================================
        preln2 = big4.tile([128, 8, S], F32, tag="a4", name="preln2")
        hT = [actp.tile([128, 8, S], BF16, tag="a2", name=f"hT{i}") for i in range(3)]
        psmm = psC
        with tc.tile_pool(name="pch", bufs=1) as pch:
            hT.append(pch.tile([128, 8, S], BF16, tag="hT3", name="hT3"))

            def ev_ffn1(mi, m, h0, ps):
                nc.scalar.activation(out=hT[m // 8][:, m % 8, h0:h0 + HALF], in_=ps,
                                     func=AF.Gelu, bias=bb("b1", m))
            proj(T["w1_blk"], range(32), 8, lambda c, h0, n: xf1_bf[:, c, h0:h0 + n],
                 ev_ffn1, pc2, "wC", psmm, "mm")

            def ev_ffn2(mi, m, h0, ps):
                nc.vector.scalar_tensor_tensor(out=preln2[:, m, h0:h0 + HALF], in0=ps,
                                               scalar=bb("b2", m),
                                               in1=xf1_f32[:, m, h0:h0 + HALF],
                                               op0=OP.add, op1=OP.add)
            proj(T["w2_blk"], range(8), 32,
                 lambda c, h0, n: hT[c // 8][:, c % 8, h0:h0 + n],
                 ev_ffn2, pc2, "wC", psmm, "mm")
        psC.release()
        pc2.release()

        # ---- LN2 -> y (fp32 to DRAM) + yT_bf ----
        yT_bf = actp.tile([128, 8, S], BF16, tag="a2", name="yT_bf")
        with tc.tile_pool(name="ystgp", bufs=2) as ystg_pool:
            ystg = [None] * 8

            def wr_ln2(c, h0, t, gkey, bkey):
                if ystg[c] is None:
                    ystg[c] = ystg_pool.tile([128, S], F32, tag="ystg", name=f"ystg{c}")
                nc.vector.tensor_scalar(out=ystg[c][:, h0:h0 + HALF], in0=t,
                                        scalar1=bb(gkey, c), scalar2=bb(bkey, c),
                                        op0=OP.mult, op1=OP.add)
                if h0 == HALVES[-1]:
                    nc.sync.dma_start(out=T["yT_out"][c * 128:(c + 1) * 128, :],
                                      in_=ystg[c])
                    nc.scalar.copy(out=yT_bf[:, c, :], in_=ystg[c])
            emit_ln(preln2, "n2g", "n2b", wr_ln2, "ln2")
        big4.release()

        # ==================================================================
        # Stage D: pattern pooling MHA + selection head
        # ==================================================================
        with tc.tile_pool(name="pd", bufs=1) as pd, \
             tc.tile_pool(name="pd2", bufs=2) as pd2, \
             tc.tile_pool(name="psDmm", bufs=2, space="PSUM") as psmm, \
             tc.tile_pool(name="psDsc", bufs=3, space="PSUM") as pssc, \
             tc.tile_pool(name="psDo", bufs=2, space="PSUM") as pso_p:

            pkT = actp.tile([128, 8, S], BF16, tag="a2", name="pkT")
            def ev_pk(mi, m, h0, ps):
                nc.vector.tensor_scalar(out=pkT[:, m, h0:h0 + HALF], in0=ps,
                                        scalar1=bb("pkb", m), scalar2=None, op0=OP.add)
            proj(T["pk_blk"], range(8), 8, lambda c, h0, n: yT_bf[:, c, h0:h0 + n],
                 ev_pk, pd2, "wD", psmm, "mm")

            pvaug = pd.tile([128, 8, H * (DH + 1)], BF16, tag="pvaug")
            for hi, h0 in enumerate(HALVES):
                pvh = pd2.tile([128, 8, HALF], BF16, tag="pvh", name=f"pvh{hi}", bufs=1)
                nc.sync.dma_start(out=pvh, in_=T["pv_pcn"][:, :, h0:h0 + HALF])
                for st in range(8):
                    ps = psmm.tile([128, HALF], F32, tag="mm", name="pspv")
                    for c in range(8):
                        nc.tensor.matmul(ps, lhsT=yT_bf[:, c, st * 128:(st + 1) * 128],
                                         rhs=pvh[:, c, :], start=(c == 0), stop=False)
                    nc.tensor.matmul(ps, lhsT=ones1,
                                     rhs=rowb[:, RB["pvb"] + h0:RB["pvb"] + h0 + HALF],
                                     start=False, stop=True)
                    nc.vector.tensor_copy(
                        out=pvaug[:, st, :].rearrange("p (h e) -> p h e", e=DH + 1)
                        [:, hi * 8:(hi + 1) * 8, 0:DH],
                        in_=ps.rearrange("p (h e) -> p h e", e=DH))
            for st in range(8):
                nc.vector.memset(
                    pvaug[:, st, :].rearrange("p (h e) -> p h e", e=DH + 1)[:, :, DH:DH + 1],
                    1.0)

            # packed bf16 bank for the small pattern/selection tensors
            # cols: queries 0:16, pqT 16:32, opat 32:48, patsTbf 48:64,
            #       selw 64:128, sqw 128:384, anw 384, gatew 385
            selbank = pd.tile([128, 8, 386], BF16, tag="selbank")
            nc.sync.dma_start(out=selbank[:, :, 0:16], in_=T["queries_pcn"])
            nc.sync.dma_start(out=selbank[:, :, 64:128], in_=T["selw_pcn"])
            nc.sync.dma_start(out=selbank[:, :, 128:384], in_=T["sqw_pcn"])
            nc.sync.dma_start(out=selbank[:, :, 384:385], in_=T["anw_pcn"])
            nc.sync.dma_start(out=selbank[:, :, 385:386], in_=T["gatew_pcn"])
            queries = selbank[:, :, 0:16]
            pqT = selbank[:, :, 16:32]
            opat = selbank[:, :, 32:48]
            patsT_bf = selbank[:, :, 48:64]
            selw = selbank[:, :, 64:128]
            sqw = selbank[:, :, 128:384]
            anw = selbank[:, :, 384:385]
            gatew = selbank[:, :, 385:386]

            # cols: skw 0:256, slotemb 256:320, skT 320:384, sqT 384:400
            skpack = pd.tile([128, 2, 400], BF16, tag="skpack")
            nc.sync.dma_start(out=skpack[:, :, 0:DC], in_=T["skw_pcn"])
            nc.sync.dma_start(out=skpack[:, :, DC:DC + KSL], in_=T["slotemb_pcn"])
            skw = skpack[:, :, 0:DC]
            slotemb = skpack[:, :, DC:DC + KSL]
            skT = skpack[:, :, 320:384]
            sqT = skpack[:, :, 384:400]

            # pattern q^T
            def ev_pq(mi, m, h0, ps):
                nc.vector.tensor_scalar(out=pqT[:, m, :], in0=ps,
                                        scalar1=bb("pqb", m), scalar2=1.0 / 8.0,
                                        op0=OP.add, op1=OP.mult)
            proj(T["pq_blk"], range(8), 8, lambda c, h0, n: queries[:, c, :],
                 ev_pq, pd2, "wD", psmm, "mm", nfree=PP, nstep=PP)

            # pattern heads: one fused scores psum + single exp per head,
            # normalization batched across all heads
            opat_un = pd.tile([128, 8, PP], BF16, tag="opat_un")
            srow = pd.tile([1, H * PP], F32, tag="srow")
            for h in range(H):
                hp = (h % 2) * 64
                hc = h // 2
                ps = pssc.tile([128, 8 * PP], F32, tag="sd", name=f"pspsc{h}")
                for kt in range(8):
                    nc.tensor.matmul(ps[:, kt * PP:(kt + 1) * PP],
                                     lhsT=pkT[hp:hp + 64, hc, kt * 128:(kt + 1) * 128],
                                     rhs=pqT[hp:hp + 64, hc, :], start=True, stop=True)
                probs = pd2.tile([128, 8 * PP], BF16, tag="pprobs", name=f"pprobs{h}")
                nc.scalar.activation(out=probs, in_=ps, func=AF.Exp)
                pso = pso_p.tile([DH + 1, PP], F32, tag="po", name="pspo")
                for kt in range(8):
                    nc.tensor.matmul(pso, lhsT=pvaug[:, kt, h * (DH + 1):(h + 1) * (DH + 1)],
                                     rhs=probs[:, kt * PP:(kt + 1) * PP],
                                     start=(kt == 0), stop=(kt == 7))
                nc.vector.tensor_copy(out=opat_un[hp:hp + 64, hc, :], in_=pso[0:DH, :])
                nc.vector.tensor_copy(out=srow[:, h * PP:(h + 1) * PP],
                                      in_=pso[DH:DH + 1, :])
            srec = pd.tile([1, H * PP], F32, tag="srec")
            nc.vector.reciprocal(out=srec, in_=srow)
            srecb = pd.tile([128, H * PP], F32, tag="srecb")
            nc.gpsimd.partition_broadcast(srecb, srec, channels=128)
            for h in range(H):
                hp = (h % 2) * 64
                hc = h // 2
                nc.vector.tensor_tensor(out=opat[hp:hp + 64, hc, :],
                                        in0=opat_un[hp:hp + 64, hc, :],
                                        in1=srecb[hp:hp + 64, h * PP:(h + 1) * PP],
                                        op=OP.mult)

            # patterns out proj (fp32 -> DRAM) + bf16 copy
            patsT = pd.tile([128, 8, PP], F32, tag="patsT")
            def ev_pwo(mi, m, h0, ps):
                nc.vector.tensor_scalar(out=patsT[:, m, :], in0=ps,
                                        scalar1=bb("pwob", m), scalar2=None, op0=OP.add)
                nc.vector.tensor_copy(out=patsT_bf[:, m, :], in_=patsT[:, m, :])
            proj(T["pwo_blk"], range(8), 8, lambda c, h0, n: opat[:, c, :],
                 ev_pwo, pd2, "wD", psmm, "mm", nfree=PP, nstep=PP)
            nc.sync.dma_start(out=T["patsT_out"].rearrange("c p n -> p c n"), in_=patsT)

            # ---- selection head ----
            # sel2d cols: learned 0:16, tt 16:32, c16 32:48, logits 48:64,
            # lg 64:80, alphabc 80:96, gT 96:112, esb0 112:176, esb1 176:240,
            # stats 240..255
            sel2d = pd.tile([128, 256], F32, tag="sel2d")
            nc.sync.dma_start(out=sel2d[0:KSL, 96:112], in_=T["gT"])
            ident = pd.tile([128, 128], F32, tag="ident")
            make_identity(nc, ident)
            sc_out = pd.tile([1, 2 * PP], F32, tag="sc_out")

            ps_s = pssc.tile([1, PP], F32, tag="sd", name="ps_s")
            for c in range(8):
                nc.tensor.matmul(ps_s, lhsT=gatew[:, c, :], rhs=patsT_bf[:, c, :],
                                 start=(c == 0), stop=(c == 7))
            nc.scalar.activation(out=sc_out[:, 0:PP], in_=ps_s, func=AF.Sigmoid,
                                 bias=bank[0:1, BB["gateb"]:BB["gateb"] + 1])
            nc.sync.dma_start(out=T["scores_out"], in_=sc_out[:, 0:PP])

            ps_a = pssc.tile([1, PP], F32, tag="sd", name="ps_a")
            for c in range(8):
                nc.tensor.matmul(ps_a, lhsT=anw[:, c, :], rhs=patsT_bf[:, c, :],
                                 start=(c == 0), stop=(c == 7))
            nc.scalar.activation(out=sc_out[:, PP:2 * PP], in_=ps_a, func=AF.Sigmoid,
                                 bias=bank[0:1, BB["anb"]:BB["anb"] + 1])
            nc.sync.dma_start(out=T["alpha_out"], in_=sc_out[:, PP:2 * PP])
            alphabc = sel2d[0:KSL, 80:96]
            nc.gpsimd.partition_broadcast(alphabc, sc_out[:, PP:2 * PP], channels=KSL)

            ps_l = pso_p.tile([KSL, PP], F32, tag="po", name="ps_l")
            for c in range(8):
                nc.tensor.matmul(ps_l, lhsT=selw[:, c, :], rhs=patsT_bf[:, c, :],
                                 start=(c == 0), stop=(c == 7))
            learned = sel2d[0:KSL, 0:16]
            nc.vector.tensor_scalar(out=learned, in0=ps_l,
                                    scalar1=bank[0:KSL, BB["selb"]:BB["selb"] + 1],
                                    scalar2=None, op0=OP.add)
            for m in range(2):
                ps = pssc.tile([128, PP], F32, tag="sd", name="ps_sq")
                for c in range(8):
                    nc.tensor.matmul(ps, lhsT=sqw[:, c, m * 128:(m + 1) * 128],
                                     rhs=patsT_bf[:, c, :], start=(c == 0), stop=(c == 7))
                nc.vector.tensor_scalar(out=sqT[:, m, :], in0=ps,
                                        scalar1=bb("sqb", m), scalar2=None, op0=OP.add)
            for m in range(2):
                ps = pssc.tile([128, KSL], F32, tag="sd", name="ps_sk")
                for c in range(2):
                    nc.tensor.matmul(ps, lhsT=skw[:, c, m * 128:(m + 1) * 128],
                                     rhs=slotemb[:, c, :], start=(c == 0), stop=(c == 1))
                nc.vector.tensor_scalar(out=skT[:, m, :], in0=ps,
                                        scalar1=bb("skb", m), scalar2=None, op0=OP.add)
            ps_c = pso_p.tile([KSL, PP], F32, tag="po", name="ps_c")
            for c in range(2):
                nc.tensor.matmul(ps_c, lhsT=skT[:, c, :], rhs=sqT[:, c, :],
                                 start=(c == 0), stop=(c == 1))
            tt = sel2d[0:KSL, 16:32]
            nc.vector.scalar_tensor_tensor(out=tt, in0=ps_c, scalar=-1.0 / 16.0,
                                           in1=learned, op0=OP.mult, op1=OP.add)
            c16 = sel2d[0:KSL, 32:48]
            nc.vector.tensor_scalar(out=c16, in0=ps_c, scalar1=1.0 / 16.0,
                                    scalar2=None, op0=OP.mult)
            logits = sel2d[0:KSL, 48:64]
            nc.vector.tensor_tensor(out=logits, in0=tt, in1=alphabc, op=OP.mult)
            nc.vector.tensor_tensor(out=logits, in0=logits, in1=c16, op=OP.add)
            lg = sel2d[0:KSL, 64:80]
            nc.vector.tensor_tensor(out=lg, in0=logits, in1=sel2d[0:KSL, 96:112],
                                    op=OP.add)

            for idx, (src, out_name) in enumerate(((lg, "slotp_out"),
                                                   (logits, "softp_out"))):
                ps_t = pssc.tile([PP, KSL], F32, tag="sd", name=f"ps_t{idx}")
                nc.tensor.transpose(ps_t, src, ident[0:KSL, 0:KSL])
                mx = sel2d[0:PP, 240 + idx * 8:241 + idx * 8]
                nc.vector.tensor_reduce(out=mx, in_=ps_t, axis=mybir.AxisListType.X,
                                        op=OP.max)
                mxn = sel2d[0:PP, 241 + idx * 8:242 + idx * 8]
                nc.vector.tensor_scalar(out=mxn, in0=mx, scalar1=-1.0, scalar2=None,
                                        op0=OP.mult)
                esb = sel2d[0:PP, 112 + idx * 64:176 + idx * 64]
                ssum = sel2d[0:PP, 242 + idx * 8:243 + idx * 8]
                nc.scalar.activation(out=esb, in_=ps_t, func=AF.Exp, bias=mxn,
                                     accum_out=ssum)
                rs = sel2d[0:PP, 243 + idx * 8:244 + idx * 8]
                nc.vector.reciprocal(out=rs, in_=ssum)
                osb = pd2.tile([PP, KSL], F32, tag="osb", name=f"osb{idx}")
                nc.vector.tensor_scalar(out=osb, in0=esb, scalar1=rs, scalar2=None,
                                        op0=OP.mult)
                nc.sync.dma_start(out=T[out_name], in_=osb)


def build_nc():
    nc = bacc.Bacc("TRN2", target_bir_lowering=False, debug=False)
    T = _declare_tensors(nc)
    with tile.TileContext(nc) as tc:
        _emit(nc, tc, T)
    nc.compile()
    return nc


# ----------------------------------------------------------------------------
# host side
# ----------------------------------------------------------------------------

def host_prep(inputs):
    f32 = np.float32
    inp = {k: np.asarray(v) for k, v in inputs.items()}

    wqkvT = inp["attn_wqkv"].T.astype(f32)
    pwT = inp["pat_wqkv"].T.astype(f32)
    decw = inp["dec_w"].astype(f32)
    decb_eff = inp["dec_b"].astype(f32) + \
        inp["layer_embed"].reshape(-1).astype(f32) @ decw[:, DC:].T

    bankv = np.zeros((128, BBW), f32)
    def setb(key, arr):
        t = _bias_t(arr)
        bankv[:, BB[key]:BB[key] + t.shape[1]] = t
    setb("cqb", inp["cq_b"]); setb("ckb", inp["ck_b"]); setb("decb", decb_eff)
    setb("fgb", inp["fg_b"]); setb("bqk", inp["attn_bqkv"][:2 * D])
    setb("wob", inp["attn_bo"]); setb("n1g", inp["n1_g"]); setb("n1b", inp["n1_b"])
    setb("n2g", inp["n2_g"]); setb("n2b", inp["n2_b"])
    setb("b1", inp["ffn_b1"]); setb("b2", inp["ffn_b2"])
    setb("pqb", inp["pat_bqkv"][:D]); setb("pkb", inp["pat_bqkv"][D:2 * D])
    setb("pwob", inp["pat_bo"]); setb("sqb", inp["sq_b"]); setb("skb", inp["sk_b"])
    bankv[0:KSL, BB["selb"]] = inp["sel_b"].astype(f32)
    bankv[0, BB["anb"]] = np.float32(inp["an_b"][0])
    bankv[0, BB["gateb"]] = np.float32(inp["gate_b"][0])
    bankv[0, BB["eps"]] = 1e-5
    bankv[:, BB["onesr"]:BB["onesr"] + 128] = 1.0

    rowv = np.zeros((1, 2304), BFNP)
    rowv[0, RB["cvb"]:RB["cvb"] + DC] = inp["cv_b"].astype(BFNP)
    rowv[0, RB["bv"]:RB["bv"] + D] = inp["attn_bqkv"][2 * D:].astype(BFNP)
    rowv[0, RB["pvb"]:RB["pvb"] + D] = inp["pat_bqkv"][2 * D:].astype(BFNP)

    shared = {
        "biasbank": bankv,
        "rowbank": rowv,
        "cqw_blk": _blk(inp["cq_w"].T),
        "ckw_pcn": _pcn(_pad_rows(inp["ck_w"].T.astype(f32), 384)),
        "cvw_pcn": _pcn(_pad_rows(inp["cv_w"].T.astype(f32), 384)),
        "decw_blk": _blk(decw[:, :DC].T),
        "fgw_blk": _blk(inp["fg_w"].T),
        "wqk_blk": _blk(wqkvT[:, :2 * D]),
        "wv_pcn": _pcn(wqkvT[:, 2 * D:]),
        "wo_blk": _blk(inp["attn_wo"].T),
        "w1_blk": _blk(inp["ffn_w1"].T),
        "w2_blk": _blk(inp["ffn_w2"].T),
        "pq_blk": _blk(pwT[:, :D]),
        "pk_blk": _blk(pwT[:, D:2 * D]),
        "pv_pcn": _pcn(pwT[:, 2 * D:]),
        "pwo_blk": _blk(inp["pat_wo"].T),
        "queries_pcn": _pcn(inp["pattern_queries"].T.astype(f32)),
        "selw_pcn": _pcn(inp["sel_w"].T.astype(f32)),
        "sqw_pcn": _pcn(inp["sq_w"].T.astype(f32)),
        "skw_pcn": _pcn(inp["sk_w"].T.astype(f32)),
        "slotemb_pcn": _pcn(inp["slot_embeddings"].T.astype(f32)),
        "anw_pcn": _pcn(inp["an_w"].T.astype(f32)),
        "gatew_pcn": _pcn(inp["gate_w"].T.astype(f32)),
    }

    g_all = -np.log(-np.log(inp["gumbel_u"].astype(f32) + 1e-8) + 1e-8)
    lids_T = inp["layer_ids"].T.astype(f32)

    in_maps = []
    for b in range(NCORES):
        cidT = np.concatenate([inp["cache"][b].T.astype(f32), lids_T], axis=0)
        m = dict(shared)
        m["xT"] = np.ascontiguousarray(inp["x"][b].T.astype(f32))
        m["cid_pcn"] = _pcn(_pad_rows(cidT, 384))
        m["gT"] = np.ascontiguousarray(g_all[b].T)
        in_maps.append(m)
    return in_maps


def _get_nc():
    if "nc" not in _CACHE:
        _CACHE["nc"] = build_nc()
    return _CACHE["nc"]


def run_on_hw(in_maps, **kw):
    nc = _get_nc()
    return bass_utils.run_bass_kernel_spmd(nc, in_maps, core_ids=list(range(NCORES)), **kw)


def assemble_outputs(results):
    y = np.stack([r["yT_out"].T for r in results])
    patterns = np.stack([r["patsT_out"].reshape(D, PP).T for r in results])
    scores = np.stack([r["scores_out"][0] for r in results])
    slot_probs = np.stack([r["slotp_out"] for r in results])
    soft_probs = np.stack([r["softp_out"] for r in results])
    alpha = np.stack([r["alpha_out"][0] for r in results])
    return (y, patterns, scores, slot_probs, soft_probs, alpha)


def kernel(**inputs):
    in_maps = host_prep(inputs)
    res = run_on_hw(in_maps)
    return assemble_outputs(res.results)
